# revision 10
# baseline (speedup 1.0000x reference)
"""Causal single-head attention (n=8192, d_model=1024, d_head=128) on 8 TRN2 cores.

Sequence-parallel, K/V projection replicated per core (collectives in this
environment have a ~100us floor -- measured -- so sharded K/V + AllGather
loses; the win over the baseline comes from everywhere else):
  - Core c owns query rows {8i + c} (mod-8 interleave -> causal work balances
    exactly and all cores run the identical SPMD instruction stream).
  - All matmul operands bf16 (full PE rate at any moving width, no fp32r
    narrow-tile 4x penalty; fp32 PSUM accumulation), halving DMA/SBUF bytes.
  - x^T streams in 16 chunks of 512 keys; K^T and V^T project per chunk;
    V^T -> V via the DMA X-bar transpose (free, overlaps compute) instead of
    64 PE transposes + DVE copies.
  - True-causal windows: for key tile J, query column m participates iff
    m >= 16J, so S/PV/Z matmuls start exactly at column 16J.  Only the
    16-wide diagonal sliver [16J, 16J+16) is partially masked, with a single
    [128,16] mask identical for every J (mask[p,o] = 8o + c >= p).
  - Attention for chunk ch-1's four key tiles interleaves with chunk ch's
    projections, so PE never waits on DMA and HAM stays warm.
  - Query columns split in two 512-halves (one PSUM bank per half for O^T/Z
    accumulators); half 0 finishes at J=31 and its normalize/project/DMA-out
    overlaps the J>=32 stream; only half 1's epilogue is tail.
  - Layouts keep everything transposed (no on-chip transposes outside the
    X-bar):
      K^T[h,j] = W_k^T x^T   (lhsT = wk k-tile,    rhs = x^T chunk)
      V^T[h,j] = W_v^T x^T   (same form), then X-bar 128x128 -> V[j,h]
      Q^T[h,m] = W_q^T x^T   (rhs = host-gathered own query columns)
      S^T[j,m] = K_J^T^T Q^T (lhsT = K^T 128-tile,  rhs = Q^T cols [16J:])
      expS     = exp(S/sqrt(d)) via ACT, PSUM -> SBUF bf16
      O^T[h,m]+= V_J^T expS  (lhsT = V j-tile,      rhs = expS)
      Z[m]    += ones^T expS (lhsT = ones[128,1],   rhs = expS)
      y[m,d]   = (O^T/Z)^T W_o  (1/Z broadcast to partitions via tiny
                                 fp32 matmul with a [1,128] ones row)
"""

import numpy as np

N_CTX = 8192
D_MODEL = 1024
D_HEAD = 128
NCORES = 8
P = 128
KT = D_MODEL // P          # 8 contraction k-tiles
R = N_CTX // NCORES        # 1024 query rows per core
NCH = 16                   # x^T chunks of 512 keys
NJ = N_CTX // P            # 64 key tiles
INV_SQRT_D = float(1.0 / np.sqrt(D_HEAD))

_CACHE = {}


def _build():
    from contextlib import ExitStack

    import concourse.mybir as mybir
    import concourse.tile as tile
    from concourse import bacc

    f32 = mybir.dt.float32
    bf16 = mybir.dt.bfloat16
    Exp = mybir.ActivationFunctionType.Exp

    nc = bacc.Bacc("TRN2", target_bir_lowering=False, debug=False,
                   num_devices=NCORES)

    # host pre-tiled inputs (see _host_in_maps)
    xt = nc.dram_tensor("xt", [NCH, P, KT, 512], bf16, kind="ExternalInput")
    xq = nc.dram_tensor("xq", [P, 2, KT, 512], bf16, kind="ExternalInput")
    wq = nc.dram_tensor("wq", [P, KT, P], bf16, kind="ExternalInput")
    wk = nc.dram_tensor("wk", [P, KT, P], bf16, kind="ExternalInput")
    wv = nc.dram_tensor("wv", [P, KT, P], bf16, kind="ExternalInput")
    wo = nc.dram_tensor("wo", [D_HEAD, D_MODEL], bf16, kind="ExternalInput")
    mask16 = nc.dram_tensor("mask16", [P, 16], bf16, kind="ExternalInput")
    ones = nc.dram_tensor("ones", [P, 1], bf16, kind="ExternalInput")
    ones1 = nc.dram_tensor("ones1", [1, P], bf16, kind="ExternalInput")
    y = nc.dram_tensor("y", [R, D_MODEL], f32, kind="ExternalOutput")

    with tile.TileContext(nc) as tc, ExitStack() as ctx:
        consts = ctx.enter_context(tc.tile_pool(name="consts", bufs=1))
        xpool = ctx.enter_context(tc.tile_pool(name="xpool", bufs=3))
        vtpool = ctx.enter_context(tc.tile_pool(name="vtpool", bufs=2))
        sepool = ctx.enter_context(tc.tile_pool(name="sepool", bufs=8))
        vpool = ctx.enter_context(tc.tile_pool(name="vpool", bufs=3))
        pslo = ctx.enter_context(tc.tile_pool(name="pslo", bufs=2, space="PSUM"))
        pshi = ctx.enter_context(tc.tile_pool(name="pshi", bufs=2, space="PSUM"))
        pacc = ctx.enter_context(tc.tile_pool(name="pacc", bufs=1, space="PSUM"))

        # ---- persistent SBUF ----
        wq_sb = consts.tile([P, KT, P], bf16, tag="wq")
        wk_sb = consts.tile([P, KT, P], bf16, tag="wk")
        wv_sb = consts.tile([P, KT, P], bf16, tag="wv")
        wo_sb = consts.tile([P, D_MODEL], bf16, tag="wo")
        mask_sb = consts.tile([P, 16], bf16, tag="mask")
        ones_sb = consts.tile([P, 1], bf16, tag="ones")
        ones1_sb = consts.tile([1, P], bf16, tag="ones1")
        xq_sb = consts.tile([P, 2, KT, 512], bf16, tag="xq")
        kT_sb = consts.tile([P, NJ, P], bf16, tag="kT")
        v_sb = consts.tile([P, NJ, P], bf16, tag="v")
        qT_sb = consts.tile([P, R], bf16, tag="qT")
        oTn_sb = consts.tile([P, R], bf16, tag="oTn")
        zr_sb = consts.tile([1, R], bf16, tag="zr")

        # ---- PSUM accumulators: O^T and Z per query-column half ----
        oT0 = pacc.tile([P, 512], f32, tag="oT0")
        oT1 = pacc.tile([P, 512], f32, tag="oT1")
        z0 = pacc.tile([1, 512], f32, tag="z0")
        z1 = pacc.tile([1, 512], f32, tag="z1")

        # ---- input DMAs ----
        nc.sync.dma_start(out=wk_sb, in_=wk[:, :, :])
        nc.sync.dma_start(out=wv_sb, in_=wv[:, :, :])
        nc.sync.dma_start(out=wq_sb, in_=wq[:, :, :])
        for ch in (0, 1):
            nc.sync.dma_start(out=xq_sb[:, ch], in_=xq[:, ch])
        nc.sync.dma_start(out=wo_sb, in_=wo[:, :])
        nc.sync.dma_start(out=mask_sb, in_=mask16[:, :])
        nc.sync.dma_start(out=ones_sb, in_=ones[:, :])
        nc.sync.dma_start(out=ones1_sb, in_=ones1[:, :])

        def load_chunk(ch):
            xt_t = xpool.tile([P, KT, 512], bf16, tag="xt", name=f"xt{ch}")
            for hf in (0, 1):
                nc.gpsimd.dma_start(
                    out=xt_t[:, 4 * hf:4 * hf + 4].rearrange(
                        "p k c -> p (k c)"),
                    in_=xt[ch, :, 4 * hf:4 * hf + 4].rearrange(
                        "p k c -> p (k c)"))
            return xt_t

        def project_chunk(ch, xt_t):
            c0 = 512 * ch
            kps = pslo.tile([P, 512], f32, tag="lo", name=f"kps{ch}")
            for kt in range(KT):
                nc.tensor.matmul(kps, wk_sb[:, kt], xt_t[:, kt],
                                 start=(kt == 0), stop=(kt == KT - 1))
            nc.vector.tensor_copy(kT_sb[:, 4 * ch:4 * ch + 4, :]
                                  .rearrange("p a b -> p (a b)"), kps)
            vps = pshi.tile([P, 512], f32, tag="hi", name=f"vps{ch}")
            for kt in range(KT):
                nc.tensor.matmul(vps, wv_sb[:, kt], xt_t[:, kt],
                                 start=(kt == 0), stop=(kt == KT - 1))
            vt_t = vtpool.tile([P, 512], bf16, tag="vt", name=f"vt{ch}")
            nc.vector.tensor_copy(vt_t, vps)
            nc.sync.dma_start(out=v_sb[:, 4 * ch:4 * ch + 4, :],
                              in_=vt_t, transpose=True)

        def project_q():
            for ch in (0, 1):
                qps = pslo.tile([P, 512], f32, tag="lo", name=f"qps{ch}")
                for kt in range(KT):
                    nc.tensor.matmul(qps, wq_sb[:, kt], xq_sb[:, ch, kt],
                                     start=(kt == 0), stop=(kt == KT - 1))
                nc.vector.tensor_copy(qT_sb[:, 512 * ch:512 * ch + 512], qps)

        def attend(J):
            m0 = 16 * J
            se = sepool.tile([P, R], bf16, tag="se", name=f"se{J}")
            if J < 32:
                slo = pslo.tile([P, 512], f32, tag="lo", name=f"slo{J}")
                shi = pshi.tile([P, 512], f32, tag="hi", name=f"shi{J}")
                nc.tensor.matmul(slo[:, m0:512], kT_sb[:, J, :],
                                 qT_sb[:, m0:512], start=True, stop=True)
                nc.tensor.matmul(shi, kT_sb[:, J, :], qT_sb[:, 512:1024],
                                 start=True, stop=True)
                nc.scalar.activation(se[:, m0:512], slo[:, m0:512], Exp,
                                     scale=INV_SQRT_D)
                nc.scalar.activation(se[:, 512:1024], shi, Exp,
                                     scale=INV_SQRT_D)
                nc.vector.tensor_mul(se[:, m0:m0 + 16], se[:, m0:m0 + 16],
                                     mask_sb)
                nc.tensor.matmul(oT0[:, m0:512], v_sb[:, J, :], se[:, m0:512],
                                 start=(J == 0), stop=(J == 31))
                nc.tensor.matmul(oT1[:, :], v_sb[:, J, :], se[:, 512:1024],
                                 start=(J == 0), stop=False)
                nc.tensor.matmul(z0[0:1, m0:512], ones_sb, se[:, m0:512],
                                 start=(J == 0), stop=(J == 31))
                nc.tensor.matmul(z1[0:1, :], ones_sb, se[:, 512:1024],
                                 start=(J == 0), stop=False)
            else:
                c0 = m0 - 512
                shi = pshi.tile([P, 512], f32, tag="hi", name=f"shi{J}")
                nc.tensor.matmul(shi[:, c0:512], kT_sb[:, J, :],
                                 qT_sb[:, m0:1024], start=True, stop=True)
                nc.scalar.activation(se[:, m0:1024], shi[:, c0:512], Exp,
                                     scale=INV_SQRT_D)
                nc.vector.tensor_mul(se[:, m0:m0 + 16], se[:, m0:m0 + 16],
                                     mask_sb)
                nc.tensor.matmul(oT1[:, c0:512], v_sb[:, J, :],
                                 se[:, m0:1024], start=False, stop=(J == 63))
                nc.tensor.matmul(z1[0:1, c0:512], ones_sb, se[:, m0:1024],
                                 start=False, stop=(J == 63))

        def normalize(h):
            oT = (oT0, oT1)[h]
            z = (z0, z1)[h]
            c0 = 512 * h
            with nc.allow_low_precision(reason="1/Z in bf16: 0.4% on the "
                                        "softmax scale, under the 2e-2 gate"):
                nc.vector.reciprocal(zr_sb[0:1, c0:c0 + 512], z[0:1, :])
            zb = pslo.tile([P, 512], f32, tag="lo", name=f"zb{h}")
            nc.tensor.matmul(zb, ones1_sb, zr_sb[0:1, c0:c0 + 512],
                             start=True, stop=True)
            zb_sb = vpool.tile([P, 512], f32, tag="y", name=f"zbs{h}")
            nc.vector.tensor_copy(zb_sb, zb)
            nc.vector.tensor_mul(oTn_sb[:, c0:c0 + 512], oT[:, :], zb_sb)

        def y_project(qt):
            pool = pslo if qt % 2 == 0 else pshi
            tg = "lo" if qt % 2 == 0 else "hi"
            for dc in range(2):
                yps = pool.tile([P, 512], f32, tag=tg, name=f"y{qt}_{dc}")
                nc.tensor.matmul(yps,
                                 oTn_sb[:, 128 * qt:128 * qt + 128],
                                 wo_sb[:, 512 * dc:512 * dc + 512],
                                 start=True, stop=True)
                y_sb = vpool.tile([P, 512], f32, tag="y",
                                  name=f"ysb{qt}_{dc}")
                nc.vector.tensor_copy(y_sb, yps)
                nc.gpsimd.dma_start(
                    out=y[128 * qt:128 * qt + 128,
                          512 * dc:512 * dc + 512],
                    in_=y_sb)

        # ---- main pipeline: project chunk ch, attend chunk ch-1 ----
        pending = load_chunk(0)
        nxt = load_chunk(1)
        for ch in range(NCH):
            xt_t = pending
            pending = nxt
            if ch + 2 < NCH:
                nxt = load_chunk(ch + 2)
            project_chunk(ch, xt_t)
            if ch == 0:
                project_q()
            else:
                for i in range(4):
                    attend(4 * (ch - 1) + i)
                if ch == 9:
                    normalize(0)
                elif 10 <= ch <= 13:
                    y_project(ch - 10)  # half-0 output, spread over chunks
        for i in range(4):
            attend(60 + i)
        normalize(1)
        for qt in range(4, 8):
            y_project(qt)

    nc.compile()
    return nc


def _get_nc():
    if "nc" not in _CACHE:
        _CACHE["nc"] = _build()
    return _CACHE["nc"]


def _host_in_maps(x, W_q, W_k, W_v, W_o):
    import ml_dtypes
    bf16 = ml_dtypes.bfloat16

    x = np.asarray(x, dtype=np.float32)
    xT = np.ascontiguousarray(x.T).astype(bf16)           # [1024, 8192]
    xTk = xT.reshape(KT, P, N_CTX)                         # [kt, p, col]
    # xt[ch, p, kt, j'] = xT[128kt+p, 512ch + j']
    xt_t = np.ascontiguousarray(
        xTk.reshape(KT, P, NCH, 512).transpose(2, 1, 0, 3))

    def wtile(w):
        return np.ascontiguousarray(
            np.asarray(w, np.float32).astype(bf16)
            .reshape(KT, P, D_HEAD).transpose(1, 0, 2))

    wq_t, wk_t, wv_t = wtile(W_q), wtile(W_k), wtile(W_v)
    wo_t = np.ascontiguousarray(np.asarray(W_o, np.float32).astype(bf16))
    ones_t = np.ones((P, 1), bf16)
    ones1_t = np.ones((1, P), bf16)
    pp = np.arange(P)[:, None]
    oo = np.arange(16)[None, :]

    in_maps = []
    for c in range(NCORES):
        # xq[p, ch, kt, m'] = xT[128kt+p, 8(512ch+m') + c]
        xq_c = np.ascontiguousarray(
            xTk[:, :, c::NCORES]
            .reshape(KT, P, 2, 512).transpose(1, 2, 0, 3))
        mask_c = (8 * oo + c >= pp).astype(bf16)           # [128, 16]
        in_maps.append({
            "xt": xt_t, "xq": xq_c,
            "wq": wq_t, "wk": wk_t, "wv": wv_t, "wo": wo_t,
            "mask16": np.ascontiguousarray(mask_c),
            "ones": ones_t, "ones1": ones1_t,
        })
    return in_maps


def _run(x, W_q, W_k, W_v, W_o, trace=False):
    from concourse.bass_utils import run_bass_kernel_spmd
    nc = _get_nc()
    in_maps = _host_in_maps(x, W_q, W_k, W_v, W_o)
    res = run_bass_kernel_spmd(nc, in_maps, list(range(NCORES)), trace=trace)
    out = np.empty((N_CTX, D_MODEL), dtype=np.float32)
    for c in range(NCORES):
        out[c::NCORES] = res.results[c]["y"]
    return out, res


def kernel(x, W_q, W_k, W_v, W_o):
    out, _ = _run(x, W_q, W_k, W_v, W_o, trace=False)
    return out


# revision 15
# speedup vs baseline: 1.0465x; 1.0465x over previous
"""Causal single-head attention (n=8192, d_model=1024, d_head=128) on 8 TRN2 cores.

Sequence-parallel, K/V projection replicated per core (collectives in this
environment have a ~100us floor -- measured -- so sharded K/V + AllGather
loses; the win over the baseline comes from everywhere else):
  - Core c owns query rows {8i + c} (mod-8 interleave -> causal work balances
    exactly and all cores run the identical SPMD instruction stream).
  - All matmul operands bf16 (full PE rate at any moving width, no fp32r
    narrow-tile 4x penalty; fp32 PSUM accumulation), halving DMA/SBUF bytes.
  - x^T streams in 16 chunks of 512 keys; K^T and V^T project per chunk;
    V^T -> V via the DMA X-bar transpose (free, overlaps compute) instead of
    64 PE transposes + DVE copies.
  - True-causal windows: for key tile J, query column m participates iff
    m >= 16J, so S/PV/Z matmuls start exactly at column 16J.  Only the
    16-wide diagonal sliver [16J, 16J+16) is partially masked, with a single
    [128,16] mask identical for every J (mask[p,o] = 8o + c >= p).
  - Attention for chunk ch-1's four key tiles interleaves with chunk ch's
    projections, so PE never waits on DMA and HAM stays warm.
  - Query columns split in two 512-halves (one PSUM bank per half for O^T/Z
    accumulators); half 0 finishes at J=31 and its normalize/project/DMA-out
    overlaps the J>=32 stream; only half 1's epilogue is tail.
  - Layouts keep everything transposed (no on-chip transposes outside the
    X-bar):
      K^T[h,j] = W_k^T x^T   (lhsT = wk k-tile,    rhs = x^T chunk)
      V^T[h,j] = W_v^T x^T   (same form), then X-bar 128x128 -> V[j,h]
      Q^T[h,m] = W_q^T x^T   (rhs = host-gathered own query columns)
      S^T[j,m] = K_J^T^T Q^T (lhsT = K^T 128-tile,  rhs = Q^T cols [16J:])
      expS     = exp(S/sqrt(d)) via ACT, PSUM -> SBUF bf16
      O^T[h,m]+= V_J^T expS  (lhsT = V j-tile,      rhs = expS)
      Z[m]    += ones^T expS (lhsT = ones[128,1],   rhs = expS)
      y[m,d]   = (O^T/Z)^T W_o  (1/Z broadcast to partitions via tiny
                                 fp32 matmul with a [1,128] ones row)
"""

import numpy as np

N_CTX = 8192
D_MODEL = 1024
D_HEAD = 128
NCORES = 8
P = 128
KT = D_MODEL // P          # 8 contraction k-tiles
R = N_CTX // NCORES        # 1024 query rows per core
NCH = 16                   # x^T chunks of 512 keys
NJ = N_CTX // P            # 64 key tiles
INV_SQRT_D = float(1.0 / np.sqrt(D_HEAD))

_CACHE = {}


def _build():
    from contextlib import ExitStack

    import concourse.mybir as mybir
    import concourse.tile as tile
    from concourse import bacc

    f32 = mybir.dt.float32
    bf16 = mybir.dt.bfloat16
    Exp = mybir.ActivationFunctionType.Exp

    nc = bacc.Bacc("TRN2", target_bir_lowering=False, debug=False,
                   num_devices=NCORES)

    # host pre-tiled inputs (see _host_in_maps)
    xt = nc.dram_tensor("xt", [NCH, P, KT, 512], bf16, kind="ExternalInput")
    xq = nc.dram_tensor("xq", [P, 2, KT, 512], bf16, kind="ExternalInput")
    wq = nc.dram_tensor("wq", [P, KT, P], bf16, kind="ExternalInput")
    wk = nc.dram_tensor("wk", [P, KT, P], bf16, kind="ExternalInput")
    wv = nc.dram_tensor("wv", [P, KT, P], bf16, kind="ExternalInput")
    wo = nc.dram_tensor("wo", [D_HEAD, D_MODEL], bf16, kind="ExternalInput")
    mask16 = nc.dram_tensor("mask16", [P, 16], bf16, kind="ExternalInput")
    ones = nc.dram_tensor("ones", [P, 1], bf16, kind="ExternalInput")
    ones1 = nc.dram_tensor("ones1", [1, P], bf16, kind="ExternalInput")
    y = nc.dram_tensor("y", [R, D_MODEL], f32, kind="ExternalOutput")

    with tile.TileContext(nc) as tc, ExitStack() as ctx:
        consts = ctx.enter_context(tc.tile_pool(name="consts", bufs=1))
        xpool = ctx.enter_context(tc.tile_pool(name="xpool", bufs=4))
        vtpool = ctx.enter_context(tc.tile_pool(name="vtpool", bufs=2))
        sepool = ctx.enter_context(tc.tile_pool(name="sepool", bufs=8))
        vpool = ctx.enter_context(tc.tile_pool(name="vpool", bufs=3))
        pslo = ctx.enter_context(tc.tile_pool(name="pslo", bufs=2, space="PSUM"))
        pshi = ctx.enter_context(tc.tile_pool(name="pshi", bufs=2, space="PSUM"))
        pacc = ctx.enter_context(tc.tile_pool(name="pacc", bufs=1, space="PSUM"))

        # ---- persistent SBUF ----
        wq_sb = consts.tile([P, KT, P], bf16, tag="wq")
        wk_sb = consts.tile([P, KT, P], bf16, tag="wk")
        wv_sb = consts.tile([P, KT, P], bf16, tag="wv")
        wo_sb = consts.tile([P, D_MODEL], bf16, tag="wo")
        mask_sb = consts.tile([P, 16], bf16, tag="mask")
        ones_sb = consts.tile([P, 1], bf16, tag="ones")
        ones1_sb = consts.tile([1, P], bf16, tag="ones1")
        xq_sb = consts.tile([P, 2, KT, 512], bf16, tag="xq")
        kT_sb = consts.tile([P, NJ, P], bf16, tag="kT")
        v_sb = consts.tile([P, NJ, P], bf16, tag="v")
        qT_sb = consts.tile([P, R], bf16, tag="qT")
        oTn_sb = consts.tile([P, R], bf16, tag="oTn")
        zr_sb = consts.tile([1, R], bf16, tag="zr")

        # ---- PSUM accumulators: O^T and Z per query-column half ----
        oT0 = pacc.tile([P, 512], f32, tag="oT0")
        oT1 = pacc.tile([P, 512], f32, tag="oT1")
        z0 = pacc.tile([1, 512], f32, tag="z0")
        z1 = pacc.tile([1, 512], f32, tag="z1")

        # ---- input DMAs ----
        nc.sync.dma_start(out=wk_sb, in_=wk[:, :, :])
        nc.sync.dma_start(out=wv_sb, in_=wv[:, :, :])
        nc.sync.dma_start(out=wq_sb, in_=wq[:, :, :])
        for ch in (0, 1):
            nc.sync.dma_start(out=xq_sb[:, ch], in_=xq[:, ch])
        nc.sync.dma_start(out=wo_sb, in_=wo[:, :])
        nc.sync.dma_start(out=mask_sb, in_=mask16[:, :])
        nc.sync.dma_start(out=ones_sb, in_=ones[:, :])
        nc.sync.dma_start(out=ones1_sb, in_=ones1[:, :])

        def load_chunk(ch):
            # halves land via both DMA paths (gpsimd SWDGE + sync HWDGE) so
            # a chunk streams in ~1.5us instead of ~3us
            xt_t = xpool.tile([P, KT, 512], bf16, tag="xt", name=f"xt{ch}")
            nc.gpsimd.dma_start(
                out=xt_t[:, 0:4].rearrange("p k c -> p (k c)"),
                in_=xt[ch, :, 0:4].rearrange("p k c -> p (k c)"))
            nc.sync.dma_start(
                out=xt_t[:, 4:8].rearrange("p k c -> p (k c)"),
                in_=xt[ch, :, 4:8].rearrange("p k c -> p (k c)"))
            return xt_t

        def project_chunk(ch, xt_t):
            kps = pslo.tile([P, 512], f32, tag="lo", name=f"kps{ch}")
            for kt in range(KT):
                nc.tensor.matmul(kps, wk_sb[:, kt], xt_t[:, kt],
                                 start=(kt == 0), stop=(kt == KT - 1))
            nc.vector.tensor_copy(kT_sb[:, 4 * ch:4 * ch + 4, :]
                                  .rearrange("p a b -> p (a b)"), kps)
            vps = pshi.tile([P, 512], f32, tag="hi", name=f"vps{ch}")
            for kt in range(KT):
                nc.tensor.matmul(vps, wv_sb[:, kt], xt_t[:, kt],
                                 start=(kt == 0), stop=(kt == KT - 1))
            vt_t = vtpool.tile([P, 512], bf16, tag="vt", name=f"vt{ch}")
            nc.vector.tensor_copy(vt_t, vps)
            nc.sync.dma_start(out=v_sb[:, 4 * ch:4 * ch + 4, :],
                              in_=vt_t, transpose=True)

        def project_q():
            for ch in (0, 1):
                qps = pslo.tile([P, 512], f32, tag="lo", name=f"qps{ch}")
                for kt in range(KT):
                    nc.tensor.matmul(qps, wq_sb[:, kt], xq_sb[:, ch, kt],
                                     start=(kt == 0), stop=(kt == KT - 1))
                nc.vector.tensor_copy(qT_sb[:, 512 * ch:512 * ch + 512], qps)

        def attend(J):
            m0 = 16 * J
            se = sepool.tile([P, R], bf16, tag="se", name=f"se{J}")
            if J < 32:
                slo = pslo.tile([P, 512], f32, tag="lo", name=f"slo{J}")
                shi = pshi.tile([P, 512], f32, tag="hi", name=f"shi{J}")
                nc.tensor.matmul(slo[:, m0:512], kT_sb[:, J, :],
                                 qT_sb[:, m0:512], start=True, stop=True)
                nc.tensor.matmul(shi, kT_sb[:, J, :], qT_sb[:, 512:1024],
                                 start=True, stop=True)
                nc.scalar.activation(se[:, m0:512], slo[:, m0:512], Exp,
                                     scale=INV_SQRT_D)
                nc.scalar.activation(se[:, 512:1024], shi, Exp,
                                     scale=INV_SQRT_D)
                nc.vector.tensor_mul(se[:, m0:m0 + 16], se[:, m0:m0 + 16],
                                     mask_sb)
                nc.tensor.matmul(oT0[:, m0:512], v_sb[:, J, :], se[:, m0:512],
                                 start=(J == 0), stop=(J == 31))
                nc.tensor.matmul(oT1[:, :], v_sb[:, J, :], se[:, 512:1024],
                                 start=(J == 0), stop=False)
                nc.tensor.matmul(z0[0:1, m0:512], ones_sb, se[:, m0:512],
                                 start=(J == 0), stop=(J == 31))
                nc.tensor.matmul(z1[0:1, :], ones_sb, se[:, 512:1024],
                                 start=(J == 0), stop=False)
            else:
                c0 = m0 - 512
                shi = pshi.tile([P, 512], f32, tag="hi", name=f"shi{J}")
                nc.tensor.matmul(shi[:, c0:512], kT_sb[:, J, :],
                                 qT_sb[:, m0:1024], start=True, stop=True)
                nc.scalar.activation(se[:, m0:1024], shi[:, c0:512], Exp,
                                     scale=INV_SQRT_D)
                nc.vector.tensor_mul(se[:, m0:m0 + 16], se[:, m0:m0 + 16],
                                     mask_sb)
                nc.tensor.matmul(oT1[:, c0:512], v_sb[:, J, :],
                                 se[:, m0:1024], start=False, stop=(J == 63))
                nc.tensor.matmul(z1[0:1, c0:512], ones_sb, se[:, m0:1024],
                                 start=False, stop=(J == 63))

        def normalize(h):
            oT = (oT0, oT1)[h]
            z = (z0, z1)[h]
            c0 = 512 * h
            with nc.allow_low_precision(reason="1/Z in bf16: 0.4% on the "
                                        "softmax scale, under the 2e-2 gate"):
                nc.vector.reciprocal(zr_sb[0:1, c0:c0 + 512], z[0:1, :])
            zb = pslo.tile([P, 512], f32, tag="lo", name=f"zb{h}")
            nc.tensor.matmul(zb, ones1_sb, zr_sb[0:1, c0:c0 + 512],
                             start=True, stop=True)
            zb_sb = vpool.tile([P, 512], f32, tag="y", name=f"zbs{h}")
            nc.vector.tensor_copy(zb_sb, zb)
            nc.vector.tensor_mul(oTn_sb[:, c0:c0 + 512], oT[:, :], zb_sb)

        def y_project(qt):
            pool = pslo if qt % 2 == 0 else pshi
            tg = "lo" if qt % 2 == 0 else "hi"
            for dc in range(2):
                yps = pool.tile([P, 512], f32, tag=tg, name=f"y{qt}_{dc}")
                nc.tensor.matmul(yps,
                                 oTn_sb[:, 128 * qt:128 * qt + 128],
                                 wo_sb[:, 512 * dc:512 * dc + 512],
                                 start=True, stop=True)
                y_sb = vpool.tile([P, 512], f32, tag="y",
                                  name=f"ysb{qt}_{dc}")
                nc.vector.tensor_copy(y_sb, yps)
                nc.gpsimd.dma_start(
                    out=y[128 * qt:128 * qt + 128,
                          512 * dc:512 * dc + 512],
                    in_=y_sb)

        # ---- main pipeline: project chunk ch, attend chunk ch-1 ----
        q = [load_chunk(0), load_chunk(1), load_chunk(2)]
        for ch in range(NCH):
            xt_t = q.pop(0)
            if ch + 3 < NCH:
                q.append(load_chunk(ch + 3))
            project_chunk(ch, xt_t)
            if ch == 0:
                project_q()
            else:
                for i in range(4):
                    attend(4 * (ch - 1) + i)
                if ch == 9:
                    normalize(0)
                elif 10 <= ch <= 13:
                    y_project(ch - 10)  # half-0 output, spread over chunks
        for i in range(4):
            attend(60 + i)
        normalize(1)
        for qt in range(4, 8):
            y_project(qt)

    nc.compile()
    return nc


def _get_nc():
    if "nc" not in _CACHE:
        _CACHE["nc"] = _build()
    return _CACHE["nc"]


def _host_in_maps(x, W_q, W_k, W_v, W_o):
    import ml_dtypes
    bf16 = ml_dtypes.bfloat16
    fp8 = ml_dtypes.float8_e4m3

    x = np.asarray(x, dtype=np.float32)
    xTf = np.ascontiguousarray(x.T)                        # [1024, 8192] f32
    xT = xTf.astype(bf16)
    xTk = xT.reshape(KT, P, N_CTX)                         # [kt, p, col]
    # xt[ch, p, kt, j'] = xT[128kt+p, 512ch + j']  (bf16, for K)
    xt_t = np.ascontiguousarray(
        xTk.reshape(KT, P, NCH, 512).transpose(2, 1, 0, 3))


    def wtile(w):
        return np.ascontiguousarray(
            np.asarray(w, np.float32).astype(bf16)
            .reshape(KT, P, D_HEAD).transpose(1, 0, 2))

    def wtile8(w):
        # [p, kt2, i, h] = W[256kt2 + 128i + p, h]  (fp8)
        return np.ascontiguousarray(
            np.asarray(w, np.float32).astype(fp8)
            .reshape(4, 2, P, D_HEAD).transpose(2, 0, 1, 3))

    wq_t, wk_t, wv_t = wtile(W_q), wtile(W_k), wtile(W_v)
    wo_t = np.ascontiguousarray(np.asarray(W_o, np.float32).astype(bf16))
    ones_t = np.ones((P, 1), bf16)
    ones1_t = np.ones((1, P), bf16)
    pp = np.arange(P)[:, None]
    oo = np.arange(16)[None, :]

    in_maps = []
    for c in range(NCORES):
        # xq[p, ch, kt, m'] = xT[128kt+p, 8(512ch+m') + c]
        xq_c = np.ascontiguousarray(
            xTk[:, :, c::NCORES]
            .reshape(KT, P, 2, 512).transpose(1, 2, 0, 3))
        mask_c = (8 * oo + c >= pp).astype(bf16)           # [128, 16]
        in_maps.append({
            "xt": xt_t, "xq": xq_c,
            "wq": wq_t, "wk": wk_t, "wv": wv_t, "wo": wo_t,
            "mask16": np.ascontiguousarray(mask_c),
            "ones": ones_t, "ones1": ones1_t,
        })
    return in_maps


def _run(x, W_q, W_k, W_v, W_o, trace=False):
    from concourse.bass_utils import run_bass_kernel_spmd
    nc = _get_nc()
    in_maps = _host_in_maps(x, W_q, W_k, W_v, W_o)
    res = run_bass_kernel_spmd(nc, in_maps, list(range(NCORES)), trace=trace)
    out = np.empty((N_CTX, D_MODEL), dtype=np.float32)
    for c in range(NCORES):
        out[c::NCORES] = res.results[c]["y"]
    return out, res


def kernel(x, W_q, W_k, W_v, W_o):
    out, _ = _run(x, W_q, W_k, W_v, W_o, trace=False)
    return out
